# revision 8
# baseline (speedup 1.0000x reference)
"""Bass/Trainium2 kernel for nn_BlockForNormalWindow (windowed-attention
transformer block), data-parallel over batch across 8 NeuronCores."""
import sys
sys.path.insert(0, '/opt/trn_rl_repo')

import numpy as np
import concourse.bass as bass
import concourse.mybir as mybir
import concourse.tile as tile
from concourse import bacc
from concourse.bass_utils import run_bass_kernel_spmd
from concourse.masks import make_identity

F32 = mybir.dt.float32
F32R = mybir.dt.float32r
AF = mybir.ActivationFunctionType
ALU = mybir.AluOpType

B, H, W = 8, 64, 64
DIM, NH, WS = 384, 6, 14
HD = DIM // NH
MLP = 4 * DIM
EPS = 1e-5
SCALE = HD ** -0.5
HP = 70
NWIN = 25
NTOK = NWIN * WS * WS        # 4900
NVAL = H * W                 # 4096
NEG = -30.0
VS = 66                      # per-head stride in v layout (64 vals + ones + pad)

PGROUPS = [(i * 392, 392) for i in range(12)] + [(4704, 196)]


def _win_origin(w):
    return (w // 5) * 14 * HP + (w % 5) * 14


def _ap(t, offset_elems, dims):
    a = t[:, 0:1]
    return bass.AP(tensor=a.tensor, offset=a.offset + offset_elems,
                   ap=[a.ap[0]] + dims)


def build_bass():
    nc = bacc.Bacc("TRN2", target_bir_lowering=False, debug=False)

    x_in = nc.dram_tensor("x", [NVAL, DIM], F32, kind="ExternalInput")
    wqk_in = nc.dram_tensor("wqk", [DIM, 2 * DIM], F32R, kind="ExternalInput")
    bqk_in = nc.dram_tensor("bqk", [2 * DIM], F32, kind="ExternalInput")
    wv_in = nc.dram_tensor("wv", [DIM, DIM], F32R, kind="ExternalInput")
    rel_in = nc.dram_tensor("rel", [HD, 2 * 196], F32R, kind="ExternalInput")
    wp_in = nc.dram_tensor("wp", [DIM, DIM], F32R, kind="ExternalInput")
    bp_in = nc.dram_tensor("bp", [DIM], F32, kind="ExternalInput")
    w1_in = nc.dram_tensor("w1", [DIM, MLP], F32R, kind="ExternalInput")
    b1_in = nc.dram_tensor("b1", [MLP], F32, kind="ExternalInput")
    w2_in = nc.dram_tensor("w2", [MLP, DIM], F32R, kind="ExternalInput")
    b2_in = nc.dram_tensor("b2", [DIM], F32, kind="ExternalInput")
    out_d = nc.dram_tensor("out", [NVAL, DIM], F32, kind="ExternalOutput")

    qT_d = nc.dram_tensor("qT_d", [NH * HD, NTOK], F32R)
    kT_d = nc.dram_tensor("kT_d", [NH * HD, NTOK], F32R)
    qrel_d = nc.dram_tensor("qrel_d", [NH, 28, NTOK], F32R)
    v_d = nc.dram_tensor("v_d", [NTOK, NH * VS], F32R)
    ind_d = nc.dram_tensor("ind_d", [2, 392], F32R)
    y_d = nc.dram_tensor("y_d", [NTOK, DIM], F32)

    with tile.TileContext(nc) as tc:
      with tc.tile_pool(name="singles", bufs=1) as singles:
        ident = singles.tile([128, 128], F32R)
        ident_f = singles.tile([128, 128], F32)
        make_identity(nc, ident_f[:])
        nc.vector.tensor_copy(out=ident[:], in_=ident_f[:])

        eps_t = singles.tile([128, 1], F32)
        nc.vector.memset(eps_t[:], EPS)

        bqk_t = singles.tile([128, 6], F32)
        nc.sync.dma_start(out=bqk_t[:], in_=bqk_in.ap().rearrange("(m p) -> p m", p=128))
        bp_t = singles.tile([128, 3], F32)
        nc.sync.dma_start(out=bp_t[:], in_=bp_in.ap().rearrange("(m p) -> p m", p=128))
        b1_t = singles.tile([128, 12], F32)
        nc.sync.dma_start(out=b1_t[:], in_=b1_in.ap().rearrange("(m p) -> p m", p=128))
        b2_t = singles.tile([128, 3], F32)
        nc.sync.dma_start(out=b2_t[:], in_=b2_in.ap().rearrange("(m p) -> p m", p=128))

        wqk_t = singles.tile([128, 3, 2 * DIM], F32R)
        nc.sync.dma_start(out=wqk_t[:], in_=wqk_in.ap().rearrange("(kc p) n -> p kc n", p=128))
        wv_t = singles.tile([128, 3, DIM], F32R)
        nc.sync.dma_start(out=wv_t[:], in_=wv_in.ap().rearrange("(kc p) n -> p kc n", p=128))
        relm_t = singles.tile([HD, 2 * 196], F32R)
        nc.sync.dma_start(out=relm_t[:], in_=rel_in.ap())
        wp_t = singles.tile([128, 3, DIM], F32R)
        nc.sync.dma_start(out=wp_t[:], in_=wp_in.ap().rearrange("(kc p) n -> p kc n", p=128))
        w1_t = singles.tile([128, 3, MLP], F32R)
        nc.sync.dma_start(out=w1_t[:], in_=w1_in.ap().rearrange("(kc p) n -> p kc n", p=128))
        w2_t = singles.tile([128, 12, DIM], F32R)
        nc.sync.dma_start(out=w2_t[:], in_=w2_in.ap().rearrange("(kc p) n -> p kc n", p=128))

        ones66 = singles.tile([1, VS], F32R)
        ones66_f = singles.tile([1, VS], F32)
        nc.vector.memset(ones66_f[:], 1.0)
        nc.vector.tensor_copy(out=ones66[:], in_=ones66_f[:])

        # k-side pattern master [64, 392]
        kpat_f = singles.tile([64, 392], F32)
        nc.vector.memset(kpat_f[:], 0.0)
        tmp_row = singles.tile([1, 392], F32)
        for j in range(14):
            nc.vector.memset(tmp_row[:], 0.0)
            nc.vector.memset(_ap(tmp_row, j * 14, [[196, 2], [1, 14]]), 1.0)
            nc.sync.dma_start(out=kpat_f[j:j + 1, :], in_=tmp_row[:])
        for j in range(14):
            nc.vector.memset(tmp_row[:], 0.0)
            nc.vector.memset(_ap(tmp_row, j, [[196, 2], [14, 14]]), 1.0)
            nc.sync.dma_start(out=kpat_f[14 + j:15 + j, :], in_=tmp_row[:])
        nc.vector.memset(tmp_row[:], 0.0)
        nc.vector.memset(tmp_row[:, 196:392], NEG)
        nc.sync.dma_start(out=kpat_f[28:29, :], in_=tmp_row[:])
        nc.vector.memset(tmp_row[:], 0.0)
        nc.vector.memset(tmp_row[:, 0:196], NEG)
        nc.sync.dma_start(out=kpat_f[29:30, :], in_=tmp_row[:])
        kpat = singles.tile([64, 392], F32R)
        nc.vector.tensor_copy(out=kpat[:], in_=kpat_f[:])

        ind_f = singles.tile([1, 392], F32)
        ind_r = singles.tile([1, 392], F32R)
        nc.vector.memset(ind_f[:], 0.0)
        nc.vector.memset(ind_f[:, 0:196], 1.0)
        nc.vector.tensor_copy(out=ind_r[:], in_=ind_f[:])
        nc.sync.dma_start(out=ind_d[0:1, :], in_=ind_r[:])
        nc.vector.memset(ind_f[:], 0.0)
        nc.vector.memset(ind_f[:, 196:392], 1.0)
        nc.vector.tensor_copy(out=ind_r[:], in_=ind_f[:])
        nc.sync.dma_start(out=ind_d[1:2, :], in_=ind_r[:])

        # ===== Phases A+B share the hT tiles =====
        with tc.tile_pool(name="pHT", bufs=1) as pHT:
            hT = [pHT.tile([128, HP * HP], F32R, tag=f"hT{c}", name=f"hT{c}") for c in range(3)]
            for c in range(3):
                nc.vector.memset(hT[c][:, 64 * HP:HP * HP].bitcast(F32), 0.0)
                nc.vector.memset(_ap(hT[c], 64, [[HP, 64], [1, 6]]).bitcast(F32), 0.0)

            # Phase A: LN1 + transpose into hT
            with tc.tile_pool(name="pA", bufs=4) as pA, \
                 tc.tile_pool(name="pA_ps", bufs=4, space="PSUM") as pA_ps:
                for t in range(32):
                    xt = pA.tile([128, DIM], F32, tag="xt")
                    nc.sync.dma_start(out=xt[:], in_=x_in[t * 128:(t + 1) * 128, :])
                    stats = pA.tile([128, 6], F32, tag="st")
                    nc.vector.bn_stats(out=stats[:], in_=xt[:])
                    mv = pA.tile([128, 2], F32, tag="mv")
                    nc.vector.bn_aggr(out=mv[:], in_=stats[:])
                    rstd = pA.tile([128, 1], F32, tag="rstd")
                    nc.scalar.activation(out=rstd[:], in_=mv[:, 1:2], func=AF.Sqrt,
                                         bias=eps_t[:], scale=1.0)
                    nc.vector.reciprocal(out=rstd[:], in_=rstd[:])
                    nmr = pA.tile([128, 1], F32, tag="nmr")
                    nc.vector.scalar_tensor_tensor(out=nmr[:], in0=mv[:, 0:1], scalar=-1.0,
                                                   in1=rstd[:], op0=ALU.mult, op1=ALU.mult)
                    hn = pA.tile([128, DIM], F32R, tag="hn")
                    nc.scalar.activation(out=hn[:], in_=xt[:], func=AF.Identity,
                                         bias=nmr[:], scale=rstd[:])
                    for c in range(3):
                        pt = pA_ps.tile([128, 128], F32R, tag="tr")
                        nc.tensor.transpose(pt[:], hn[:, c * 128:(c + 1) * 128], ident[:])
                        nc.scalar.copy(out=_ap(hT[c], 2 * t * HP, [[HP, 2], [1, 64]]),
                                       in_=pt[:])

            # Phase B: qkv
            with tc.tile_pool(name="pB", bufs=4) as pB, \
                 tc.tile_pool(name="pB_ps", bufs=3, space="PSUM") as pB_ps, \
                 tc.tile_pool(name="pBv_ps", bufs=3, space="PSUM") as pBv_ps:
                for (p0, plen) in PGROUPS:
                    w0 = (p0 // 392) * 2
                    nwin = 2 if plen == 392 else 1
                    o0 = _win_origin(w0)
                    if nwin == 2:
                        dims = [[_win_origin(w0 + 1) - o0, 2], [HP, 14], [1, 14]]
                    else:
                        dims = [[HP, 14], [1, 14]]
                    for m in range(6):
                        ps = pB_ps.tile([128, 392], F32, tag="qk_ps")
                        for kc in range(3):
                            nc.tensor.matmul(ps[:, 0:plen],
                                             wqk_t[:, kc, m * 128:(m + 1) * 128],
                                             _ap(hT[kc], o0, dims),
                                             start=(kc == 0), stop=(kc == 2))
                        sb = pB.tile([128, 392], F32R, tag="qk_sb")
                        nc.scalar.activation(out=sb[:, 0:plen], in_=ps[:, 0:plen],
                                             func=AF.Identity, bias=bqk_t[:, m:m + 1],
                                             scale=1.0)
                        dst = qT_d if m < 3 else kT_d
                        mm = m % 3
                        nc.sync.dma_start(out=dst[mm * 128:(mm + 1) * 128, p0:p0 + plen],
                                          in_=sb[:, 0:plen])
                    for s in range(plen // 98):
                        w = w0 + (98 * s) // 196
                        r0 = ((98 * s) % 196) // 14
                        ov = _win_origin(w) + r0 * HP
                        ps = pBv_ps.tile([98, DIM], F32, tag="v_ps")
                        hstage = pB.tile([128, 3, 98], F32R, tag="hstage")
                        for kc in range(3):
                            nc.gpsimd.tensor_copy(
                                out=hstage[:, kc, :],
                                in_=_ap(hT[kc], ov, [[HP, 7], [1, 14]]))
                        for kc in range(3):
                            nc.tensor.matmul(ps[:], hstage[:, kc, :], wv_t[:, kc, :],
                                             start=(kc == 0), stop=(kc == 2))
                        sb = pB.tile([98, NH * VS], F32R, tag="v_sb")
                        nc.scalar.copy(out=_ap(sb, 0, [[VS, 6], [1, 64]]), in_=ps[:])
                        nc.vector.memset(_ap(sb, 64, [[VS, 6], [1, 2]]).bitcast(F32), 0.0)
                        nc.vector.memset(_ap(sb, 64, [[VS, 6], [1, 1]]).bitcast(F32), 1.0)
                        nc.sync.dma_start(out=v_d[p0 + 98 * s:p0 + 98 * s + 98, :],
                                          in_=sb[:])

        # ===== Phase C: rel values =====
        with tc.tile_pool(name="pC", bufs=2) as pC, \
             tc.tile_pool(name="pC_ps", bufs=4, space="PSUM") as pC_ps:
            for h in range(NH):
                qTh = pC.tile([HD, NTOK], F32R, tag="qTh")
                nc.sync.dma_start(out=qTh[:], in_=qT_d[h * HD:(h + 1) * HD, :])
                relh_sb = pC.tile([14, NTOK], F32R, tag="relh")
                relw_sb = pC.tile([14, NTOK], F32R, tag="relw")
                for r in range(14):
                    ps = pC_ps.tile([14, 350], F32, tag="rel_ps")
                    nc.tensor.matmul(ps[:], relm_t[:, r * 14:(r + 1) * 14],
                                     _ap(qTh, r * 14, [[196, 25], [1, 14]]),
                                     start=True, stop=True)
                    dst = _ap(relh_sb, r * 14, [[196, 25], [1, 14]])
                    if r % 2 == 0:
                        nc.scalar.copy(out=dst, in_=ps[:])
                    else:
                        nc.vector.tensor_copy(out=dst, in_=ps[:])
                for c in range(14):
                    ps = pC_ps.tile([14, 350], F32, tag="rel_ps")
                    nc.tensor.matmul(ps[:], relm_t[:, 196 + c * 14:196 + (c + 1) * 14],
                                     _ap(qTh, c, [[196, 25], [14, 14]]),
                                     start=True, stop=True)
                    dst = _ap(relw_sb, c, [[196, 25], [14, 14]])
                    if c % 2 == 0:
                        nc.scalar.copy(out=dst, in_=ps[:])
                    else:
                        nc.vector.tensor_copy(out=dst, in_=ps[:])
                nc.sync.dma_start(out=qrel_d[h, 0:14, :], in_=relh_sb[:])
                nc.sync.dma_start(out=qrel_d[h, 14:28, :], in_=relw_sb[:])

        # ===== Phase D: attention + proj =====
        with tc.tile_pool(name="pD", bufs=3) as pD, \
             tc.tile_pool(name="pDv", bufs=2) as pDv, \
             tc.tile_pool(name="pDa", bufs=2) as pDa, \
             tc.tile_pool(name="pDy", bufs=2) as pDy, \
             tc.tile_pool(name="pDe", bufs=6) as pDe, \
             tc.tile_pool(name="pDs_ps", bufs=2, space="PSUM") as pDs_ps, \
             tc.tile_pool(name="pDo_ps", bufs=2, space="PSUM") as pDo_ps, \
             tc.tile_pool(name="pDz_ps", bufs=1, space="PSUM") as pDz_ps, \
             tc.tile_pool(name="pDp_ps", bufs=1, space="PSUM") as pDp_ps:
            for (p0, plen) in PGROUPS:
                nk = plen // 98
                vt = [pDv.tile([98, NH * VS], F32R, tag=f"vt{s}", name=f"vt{s}") for s in range(nk)]
                for s in range(nk):
                    nc.sync.dma_start(out=vt[s][:],
                                      in_=v_d[p0 + 98 * s:p0 + 98 * (s + 1), :])
                attnT = [pDa.tile([128, 392], F32R, tag=f"attnT{m}", name=f"attnT{m}") for m in range(3)]
                for h in range(NH):
                    kTa = pD.tile([128, 392], F32R, tag="kTa")
                    nc.sync.dma_start(out=kTa[0:64, 0:plen],
                                      in_=kT_d[h * HD:(h + 1) * HD, p0:p0 + plen])
                    nc.gpsimd.tensor_copy(out=kTa[64:128, 0:plen], in_=kpat[:, 0:plen])
                    qTa = pD.tile([128, 392], F32R, tag="qTa")
                    nc.sync.dma_start(out=qTa[0:64, 0:plen],
                                      in_=qT_d[h * HD:(h + 1) * HD, p0:p0 + plen])
                    nc.sync.dma_start(out=qTa[64:92, 0:plen],
                                      in_=qrel_d[h, :, p0:p0 + plen])
                    nc.sync.dma_start(out=qTa[92:94, 0:plen], in_=ind_d[:, 0:plen])

                    oT = pDo_ps.tile([VS, 392], F32, tag="oT")
                    for s in range(nk):
                        st = pDs_ps.tile([98, 392], F32, tag="st")
                        nc.tensor.matmul(st[:, 0:plen], kTa[:, s * 98:(s + 1) * 98],
                                         qTa[:, 0:plen], start=True, stop=True)
                        et = pDe.tile([98, 392], F32R, tag="et")
                        nc.scalar.activation(out=et[:, 0:plen], in_=st[:, 0:plen],
                                             func=AF.Exp, bias=0.0, scale=1.0)
                        nc.tensor.matmul(oT[:, 0:plen], vt[s][:, h * VS:(h + 1) * VS],
                                         et[:, 0:plen], start=(s == 0),
                                         stop=(s == nk - 1))
                    rz = pD.tile([1, 392], F32R, tag="rz")
                    with nc.allow_low_precision(reason="recip feeds fp32r bcast matmul"):
                        nc.vector.reciprocal(out=rz[:, 0:plen], in_=oT[64:65, 0:plen])
                    zb = pDz_ps.tile([VS, 392], F32, tag="zb")
                    nc.tensor.matmul(zb[:, 0:plen], ones66[:], rz[:, 0:plen],
                                     start=True, stop=True)
                    ot_sb = pD.tile([64, 392], F32, tag="ot_sb")
                    nc.scalar.copy(out=ot_sb[:, 0:plen], in_=oT[0:64, 0:plen])
                    nc.vector.tensor_tensor(
                        out=attnT[h // 2][(h % 2) * 64:(h % 2) * 64 + 64, 0:plen],
                        in0=ot_sb[:, 0:plen], in1=zb[0:64, 0:plen], op=ALU.mult)
                yT = []
                for m in range(3):
                    pj = pDp_ps.tile([128, 392], F32, tag="pj")
                    for kc in range(3):
                        nc.tensor.matmul(pj[:, 0:plen],
                                         wp_t[:, kc, m * 128:(m + 1) * 128],
                                         attnT[kc][:, 0:plen],
                                         start=(kc == 0), stop=(kc == 2))
                    yTm = pDy.tile([128, 392], F32R, tag=f"yT{m}")
                    nc.scalar.activation(out=yTm[:, 0:plen], in_=pj[:, 0:plen],
                                         func=AF.Identity, bias=bp_t[:, m:m + 1],
                                         scale=1.0)
                    yT.append(yTm)
                for s in range(nk):
                    pt = pDp_ps.tile([98, DIM], F32R, tag="ytr")
                    for m in range(3):
                        nc.tensor.transpose(pt[:, m * 128:(m + 1) * 128],
                                            yT[m][:, s * 98:(s + 1) * 98], ident[:])
                    ysb = pD.tile([98, DIM], F32, tag="ysb")
                    nc.vector.tensor_copy(out=ysb[:], in_=pt[:].bitcast(F32))
                    nc.sync.dma_start(out=y_d[p0 + 98 * s:p0 + 98 * (s + 1), :],
                                      in_=ysb[:])

        # ===== Phase E: residual + LN2 + MLP =====
        with tc.tile_pool(name="pE", bufs=3) as pE, \
             tc.tile_pool(name="pEz", bufs=6) as pEz, \
             tc.tile_pool(name="pEh", bufs=2) as pEh, \
             tc.tile_pool(name="pEgG", bufs=2) as pEgG, \
             tc.tile_pool(name="pE_ps", bufs=2, space="PSUM") as pE_ps, \
             tc.tile_pool(name="pE2_ps", bufs=2, space="PSUM") as pE2_ps, \
             tc.tile_pool(name="pE3_ps", bufs=2, space="PSUM") as pE3_ps:
            for g in range(8):
                zts = []
                h2T = [pEh.tile([128, 512], F32R, tag=f"h2T{c}", name=f"h2T{c}") for c in range(3)]
                for tt in range(4):
                    t = g * 4 + tt
                    xt = pE.tile([128, DIM], F32, tag="xe")
                    nc.sync.dma_start(out=xt[:], in_=x_in[t * 128:(t + 1) * 128, :])
                    yt = pE.tile([128, DIM], F32, tag="ye")
                    ysrc = y_d.ap()[0:1, :]
                    for rr in range(2):
                        i = 2 * t + rr
                        wi, r = i // 14, i % 14
                        pbase = (wi * 5) * 196 + r * 14
                        nc.sync.dma_start(
                            out=yt[rr * 64:rr * 64 + 56, :],
                            in_=bass.AP(tensor=ysrc.tensor,
                                        offset=ysrc.offset + pbase * DIM,
                                        ap=[[196 * DIM, 4], [DIM, 14], [1, DIM]]))
                        nc.sync.dma_start(
                            out=yt[rr * 64 + 56:rr * 64 + 64, :],
                            in_=bass.AP(tensor=ysrc.tensor,
                                        offset=ysrc.offset + (pbase + 4 * 196) * DIM,
                                        ap=[[DIM, 8], [1, DIM]]))
                    zt = pEz.tile([128, DIM], F32, tag="ze")
                    nc.vector.tensor_tensor(out=zt[:], in0=xt[:], in1=yt[:], op=ALU.add)
                    zts.append(zt)
                    stats = pE.tile([128, 6], F32, tag="st_e")
                    nc.vector.bn_stats(out=stats[:], in_=zt[:])
                    mv = pE.tile([128, 2], F32, tag="mv_e")
                    nc.vector.bn_aggr(out=mv[:], in_=stats[:])
                    rstd = pE.tile([128, 1], F32, tag="rstd_e")
                    nc.scalar.activation(out=rstd[:], in_=mv[:, 1:2], func=AF.Sqrt,
                                         bias=eps_t[:], scale=1.0)
                    nc.vector.reciprocal(out=rstd[:], in_=rstd[:])
                    nmr = pE.tile([128, 1], F32, tag="nmr_e")
                    nc.vector.scalar_tensor_tensor(out=nmr[:], in0=mv[:, 0:1],
                                                   scalar=-1.0, in1=rstd[:],
                                                   op0=ALU.mult, op1=ALU.mult)
                    hn = pE.tile([128, DIM], F32R, tag="hn_e")
                    nc.scalar.activation(out=hn[:], in_=zt[:], func=AF.Identity,
                                         bias=nmr[:], scale=rstd[:])
                    for c in range(3):
                        pt = pE_ps.tile([128, 128], F32R, tag="htr")
                        nc.tensor.transpose(pt[:], hn[:, c * 128:(c + 1) * 128], ident[:])
                        nc.scalar.copy(out=h2T[c][:, tt * 128:(tt + 1) * 128], in_=pt[:])
                gt = []
                for m in range(12):
                    ps = pE2_ps.tile([128, 512], F32, tag="fc1")
                    for kc in range(3):
                        nc.tensor.matmul(ps[:], w1_t[:, kc, m * 128:(m + 1) * 128],
                                         h2T[kc][:], start=(kc == 0), stop=(kc == 2))
                    gm = pEgG.tile([128, 512], F32R, tag=f"g{m}")
                    nc.scalar.activation(out=gm[:], in_=ps[:], func=AF.Gelu,
                                         bias=b1_t[:, m:m + 1], scale=1.0)
                    gt.append(gm)
                o2T = []
                for m in range(3):
                    ps = pE3_ps.tile([128, 512], F32, tag="fc2")
                    for kc in range(12):
                        nc.tensor.matmul(ps[:], w2_t[:, kc, m * 128:(m + 1) * 128],
                                         gt[kc][:], start=(kc == 0), stop=(kc == 11))
                    om = pEh.tile([128, 512], F32R, tag=f"o2T{m}")
                    nc.scalar.activation(out=om[:], in_=ps[:], func=AF.Identity,
                                         bias=b2_t[:, m:m + 1], scale=1.0)
                    o2T.append(om)
                for tt in range(4):
                    t = g * 4 + tt
                    pt = pE_ps.tile([128, DIM], F32R, tag="otr")
                    for c in range(3):
                        nc.tensor.transpose(pt[:, c * 128:(c + 1) * 128],
                                            o2T[c][:, tt * 128:(tt + 1) * 128], ident[:])
                    ot = pE.tile([128, DIM], F32, tag="oe")
                    nc.vector.tensor_tensor(out=ot[:], in0=zts[tt][:], in1=pt[:].bitcast(F32),
                                            op=ALU.add)
                    nc.sync.dma_start(out=out_d[t * 128:(t + 1) * 128, :], in_=ot[:])

    nc.compile()
    return nc


_NC = None


def _get_nc():
    global _NC
    if _NC is None:
        _NC = build_bass()
    return _NC


def _host_prep(inputs):
    f = np.float32
    ln1_w = np.asarray(inputs["ln1_w"], f); ln1_b = np.asarray(inputs["ln1_b"], f)
    qkv_w = np.asarray(inputs["qkv_w"], f); qkv_b = np.asarray(inputs["qkv_b"], f)
    proj_w = np.asarray(inputs["proj_w"], f); proj_b = np.asarray(inputs["proj_b"], f)
    ln2_w = np.asarray(inputs["ln2_w"], f); ln2_b = np.asarray(inputs["ln2_b"], f)
    fc1_w = np.asarray(inputs["fc1_w"], f); fc1_b = np.asarray(inputs["fc1_b"], f)
    fc2_w = np.asarray(inputs["fc2_w"], f); fc2_b = np.asarray(inputs["fc2_b"], f)
    rel_h = np.asarray(inputs["rel_pos_h"], f); rel_w = np.asarray(inputs["rel_pos_w"], f)

    wqk = (ln1_w[:, None] * qkv_w[:, :768]).copy()
    bqk = (ln1_b @ qkv_w[:, :768] + qkv_b[:768]).copy()
    wqk[:, :384] *= SCALE
    bqk[:384] *= SCALE
    wv = (ln1_w[:, None] * qkv_w[:, 768:]).copy()
    bv = ln1_b @ qkv_w[:, 768:] + qkv_b[768:]

    coords = np.arange(WS)[:, None] - np.arange(WS)[None, :] + (WS - 1)
    Rh = rel_h[coords]
    Rw = rel_w[coords]
    rel = np.zeros((HD, 2 * 196), f)
    for r in range(14):
        rel[:, r * 14:(r + 1) * 14] = Rh[r].T / SCALE
    for c in range(14):
        rel[:, 196 + c * 14:196 + (c + 1) * 14] = Rw[c].T / SCALE

    return {
        "wqk": np.ascontiguousarray(wqk, f), "bqk": np.ascontiguousarray(bqk, f),
        "wv": np.ascontiguousarray(wv, f), "rel": rel,
        "wp": np.ascontiguousarray(proj_w, f),
        "bp": np.ascontiguousarray(proj_b + bv @ proj_w, f),
        "w1": np.ascontiguousarray(ln2_w[:, None] * fc1_w, f),
        "b1": np.ascontiguousarray(ln2_b @ fc1_w + fc1_b, f),
        "w2": np.ascontiguousarray(fc2_w, f),
        "b2": np.ascontiguousarray(fc2_b, f),
    }


def kernel(**inputs):
    nc = _get_nc()
    shared = _host_prep(inputs)
    x = np.asarray(inputs["x"], np.float32).reshape(B, NVAL, DIM)
    in_maps = [dict(shared, x=np.ascontiguousarray(x[c])) for c in range(B)]
    res = run_bass_kernel_spmd(nc, in_maps, list(range(B)))
    out = np.stack([res.results[c]["out"] for c in range(B)])
    return out.reshape(B, H, W, DIM)


if __name__ == "__main__":
    build_bass()
    print("build ok")


# revision 12
# speedup vs baseline: 90.9804x; 90.9804x over previous
"""Bass/Trainium2 kernel for nn_BlockForNormalWindow (windowed-attention
transformer block), data-parallel over batch across 8 NeuronCores."""
import sys
sys.path.insert(0, '/opt/trn_rl_repo')

import numpy as np
import concourse.bass as bass
import concourse.mybir as mybir
import concourse.tile as tile
from concourse import bacc
from concourse.bass_utils import run_bass_kernel_spmd
from concourse.masks import make_identity

F32 = mybir.dt.float32
F32R = mybir.dt.float32r
AF = mybir.ActivationFunctionType
ALU = mybir.AluOpType

B, H, W = 8, 64, 64
DIM, NH, WS = 384, 6, 14
HD = DIM // NH
MLP = 4 * DIM
EPS = 1e-5
SCALE = HD ** -0.5
HP = 70
NWIN = 25
NTOK = NWIN * WS * WS        # 4900
NVAL = H * W                 # 4096
NEG = -30.0
VS = 66                      # per-head stride in v layout (64 vals + ones + pad)

PGROUPS = [(i * 392, 392) for i in range(12)] + [(4704, 196)]


def _win_origin(w):
    return (w // 5) * 14 * HP + (w % 5) * 14


def _ap(t, offset_elems, dims):
    a = t[:, 0:1]
    return bass.AP(tensor=a.tensor, offset=a.offset + offset_elems,
                   ap=[a.ap[0]] + dims)


def build_bass():
    nc = bacc.Bacc("TRN2", target_bir_lowering=False, debug=False)

    x_in = nc.dram_tensor("x", [NVAL, DIM], F32, kind="ExternalInput")
    wqk_in = nc.dram_tensor("wqk", [DIM, 2 * DIM], F32R, kind="ExternalInput")
    bqk_in = nc.dram_tensor("bqk", [2 * DIM], F32, kind="ExternalInput")
    wv_in = nc.dram_tensor("wv", [DIM, DIM], F32R, kind="ExternalInput")
    rel_in = nc.dram_tensor("rel", [HD, 2 * 196], F32R, kind="ExternalInput")
    wp_in = nc.dram_tensor("wp", [DIM, DIM], F32R, kind="ExternalInput")
    bp_in = nc.dram_tensor("bp", [DIM], F32, kind="ExternalInput")
    w1_in = nc.dram_tensor("w1", [DIM, MLP], F32R, kind="ExternalInput")
    b1_in = nc.dram_tensor("b1", [MLP], F32, kind="ExternalInput")
    w2_in = nc.dram_tensor("w2", [MLP, DIM], F32R, kind="ExternalInput")
    b2_in = nc.dram_tensor("b2", [DIM], F32, kind="ExternalInput")
    out_d = nc.dram_tensor("out", [NVAL, DIM], F32, kind="ExternalOutput")

    kT_d = nc.dram_tensor("kT_d", [NH * HD, NTOK], F32R)
    qrel_d = nc.dram_tensor("qrel_d", [NH, 94, NTOK], F32R)
    v_d = nc.dram_tensor("v_d", [NTOK, NH * VS], F32R)
    y_d = nc.dram_tensor("y_d", [HP * HP, DIM], F32)

    with tile.TileContext(nc) as tc:
      with tc.tile_pool(name="singles", bufs=1) as singles:
        ident = singles.tile([128, 128], F32R)
        ident_f = singles.tile([128, 128], F32)
        make_identity(nc, ident_f[:])
        nc.vector.tensor_copy(out=ident[:], in_=ident_f[:])

        eps_t = singles.tile([128, 1], F32)
        nc.vector.memset(eps_t[:], EPS)

        bqk_t = singles.tile([128, 6], F32)
        nc.sync.dma_start(out=bqk_t[:], in_=bqk_in.ap().rearrange("(m p) -> p m", p=128))
        bp_t = singles.tile([128, 3], F32)
        nc.sync.dma_start(out=bp_t[:], in_=bp_in.ap().rearrange("(m p) -> p m", p=128))
        b1_t = singles.tile([128, 12], F32)
        nc.sync.dma_start(out=b1_t[:], in_=b1_in.ap().rearrange("(m p) -> p m", p=128))
        b2_t = singles.tile([128, 3], F32)
        nc.sync.dma_start(out=b2_t[:], in_=b2_in.ap().rearrange("(m p) -> p m", p=128))

        wqk_t = singles.tile([128, 3, 2 * DIM], F32R)
        nc.sync.dma_start(out=wqk_t[:], in_=wqk_in.ap().rearrange("(kc p) n -> p kc n", p=128))
        wv_t = singles.tile([128, 3, DIM], F32R)
        nc.sync.dma_start(out=wv_t[:], in_=wv_in.ap().rearrange("(kc p) n -> p kc n", p=128))
        relm_t = singles.tile([HD, 2 * 196], F32R)
        nc.sync.dma_start(out=relm_t[:], in_=rel_in.ap())
        wp_t = singles.tile([128, 3, DIM], F32R)
        nc.sync.dma_start(out=wp_t[:], in_=wp_in.ap().rearrange("(kc p) n -> p kc n", p=128))
        w1_t = singles.tile([128, 3, MLP], F32R)
        nc.sync.dma_start(out=w1_t[:], in_=w1_in.ap().rearrange("(kc p) n -> p kc n", p=128))
        w2_t = singles.tile([128, 12, DIM], F32R)
        nc.sync.dma_start(out=w2_t[:], in_=w2_in.ap().rearrange("(kc p) n -> p kc n", p=128))

        # k-side pattern master [64, 392]
        kpat_f = singles.tile([64, 392], F32)
        nc.vector.memset(kpat_f[:], 0.0)
        tmp_row = singles.tile([1, 392], F32)
        for j in range(14):
            nc.vector.memset(tmp_row[:], 0.0)
            nc.vector.memset(_ap(tmp_row, j * 14, [[196, 2], [1, 14]]), 1.0)
            nc.sync.dma_start(out=kpat_f[j:j + 1, :], in_=tmp_row[:])
        for j in range(14):
            nc.vector.memset(tmp_row[:], 0.0)
            nc.vector.memset(_ap(tmp_row, j, [[196, 2], [14, 14]]), 1.0)
            nc.sync.dma_start(out=kpat_f[14 + j:15 + j, :], in_=tmp_row[:])
        nc.vector.memset(tmp_row[:], 0.0)
        nc.vector.memset(tmp_row[:, 196:392], NEG)
        nc.sync.dma_start(out=kpat_f[28:29, :], in_=tmp_row[:])
        nc.vector.memset(tmp_row[:], 0.0)
        nc.vector.memset(tmp_row[:, 0:196], NEG)
        nc.sync.dma_start(out=kpat_f[29:30, :], in_=tmp_row[:])
        kpat = singles.tile([64, 392], F32R)
        nc.vector.tensor_copy(out=kpat[:], in_=kpat_f[:])

        indA = singles.tile([1, 392], F32R)
        indB = singles.tile([1, 392], F32R)
        nc.gpsimd.memset(indA[:].bitcast(F32), 0.0)
        nc.gpsimd.memset(indA[:, 0:196].bitcast(F32), 1.0)
        nc.gpsimd.memset(indB[:].bitcast(F32), 0.0)
        nc.gpsimd.memset(indB[:, 196:392].bitcast(F32), 1.0)

        # ===== Phases A+B share the hT tiles =====
        with tc.tile_pool(name="pHT", bufs=1) as pHT:
            hT = [pHT.tile([128, HP * HP], F32R, tag=f"hT{c}", name=f"hT{c}") for c in range(3)]
            for c in range(3):
                nc.gpsimd.memset(hT[c][:, 64 * HP:HP * HP].bitcast(F32), 0.0)
                nc.gpsimd.memset(_ap(hT[c], 64, [[HP, 64], [1, 6]]).bitcast(F32), 0.0)

            # Phase A: LN1 + transpose into hT
            with tc.tile_pool(name="pA", bufs=4) as pA, \
                 tc.tile_pool(name="pA_ps", bufs=4, space="PSUM") as pA_ps:
                for t in range(32):
                    xt = pA.tile([128, DIM], F32, tag="xt")
                    nc.sync.dma_start(out=xt[:], in_=x_in[t * 128:(t + 1) * 128, :])
                    stats = pA.tile([128, 6], F32, tag="st")
                    nc.vector.bn_stats(out=stats[:], in_=xt[:])
                    mv = pA.tile([128, 2], F32, tag="mv")
                    nc.vector.bn_aggr(out=mv[:], in_=stats[:])
                    rstd = pA.tile([128, 1], F32, tag="rstd")
                    nc.scalar.activation(out=rstd[:], in_=mv[:, 1:2], func=AF.Sqrt,
                                         bias=eps_t[:], scale=1.0)
                    nc.vector.reciprocal(out=rstd[:], in_=rstd[:])
                    nmr = pA.tile([128, 1], F32, tag="nmr")
                    nc.vector.tensor_scalar_mul(out=nmr[:], in0=mv[:, 0:1], scalar1=-1.0)
                    hn = pA.tile([128, DIM], F32R, tag="hn")
                    nc.scalar.activation(out=hn[:], in_=xt[:], func=AF.Identity,
                                         bias=nmr[:], scale=rstd[:]) if False else \
                    nc.vector.tensor_scalar(out=hn[:], in0=xt[:], scalar1=nmr[:],
                                            scalar2=rstd[:], op0=ALU.add, op1=ALU.mult)
                    for c in range(3):
                        pt = pA_ps.tile([128, 128], F32R, tag="tr")
                        nc.tensor.transpose(pt[:], hn[:, c * 128:(c + 1) * 128], ident[:])
                        dst = _ap(hT[c], 2 * t * HP, [[HP, 2], [1, 64]])
                        if (t * 3 + c) % 2 == 0:
                            nc.scalar.copy(out=dst, in_=pt[:])
                        else:
                            nc.vector.tensor_copy(out=dst, in_=pt[:])

            # Phase B: qkv
            with tc.tile_pool(name="pB", bufs=4) as pB, \
                 tc.tile_pool(name="pB_ps", bufs=3, space="PSUM") as pB_ps, \
                 tc.tile_pool(name="pBv_ps", bufs=3, space="PSUM") as pBv_ps:
                for (p0, plen) in PGROUPS:
                    w0 = (p0 // 392) * 2
                    nwin = 2 if plen == 392 else 1
                    o0 = _win_origin(w0)
                    if nwin == 2:
                        dims = [[_win_origin(w0 + 1) - o0, 2], [HP, 14], [1, 14]]
                    else:
                        dims = [[HP, 14], [1, 14]]
                    for m in range(6):
                        ps = pB_ps.tile([128, 392], F32, tag="qk_ps")
                        for kc in range(3):
                            nc.tensor.matmul(ps[:, 0:plen],
                                             wqk_t[:, kc, m * 128:(m + 1) * 128],
                                             _ap(hT[kc], o0, dims),
                                             start=(kc == 0), stop=(kc == 2))
                        sb = pB.tile([128, 392], F32R, tag="qk_sb")
                        if m % 2 == 0:
                            nc.scalar.activation(out=sb[:, 0:plen], in_=ps[:, 0:plen],
                                                 func=AF.Identity, bias=bqk_t[:, m:m + 1],
                                                 scale=1.0)
                        else:
                            nc.vector.tensor_scalar(out=sb[:, 0:plen], in0=ps[:, 0:plen],
                                                    scalar1=bqk_t[:, m:m + 1], scalar2=None,
                                                    op0=ALU.add)
                        if m < 3:
                            qr = qrel_d.ap()
                            dst_ap = bass.AP(
                                tensor=qr.tensor,
                                offset=2 * m * 94 * NTOK + p0,
                                ap=[[94 * NTOK, 2], [NTOK, 64], [1, plen]])
                            nc.sync.dma_start(out=dst_ap, in_=sb[:, 0:plen])
                        else:
                            mm = m - 3
                            nc.sync.dma_start(
                                out=kT_d[mm * 128:(mm + 1) * 128, p0:p0 + plen],
                                in_=sb[:, 0:plen])
                    for s in range(plen // 98):
                        w = w0 + (98 * s) // 196
                        r0 = ((98 * s) % 196) // 14
                        ov = _win_origin(w) + r0 * HP
                        ps = pBv_ps.tile([98, DIM], F32, tag="v_ps")
                        hstage = pB.tile([128, 3, 98], F32R, tag="hstage")
                        for kc in range(3):
                            nc.gpsimd.tensor_copy(
                                out=hstage[:, kc, :],
                                in_=_ap(hT[kc], ov, [[HP, 7], [1, 14]]))
                        for kc in range(3):
                            nc.tensor.matmul(ps[:], hstage[:, kc, :], wv_t[:, kc, :],
                                             start=(kc == 0), stop=(kc == 2))
                        sb = pB.tile([98, NH * VS], F32R, tag="v_sb")
                        if s % 2 == 0:
                            nc.scalar.copy(out=_ap(sb, 0, [[VS, 6], [1, 64]]), in_=ps[:])
                        else:
                            nc.vector.tensor_copy(out=_ap(sb, 0, [[VS, 6], [1, 64]]),
                                                  in_=ps[:])
                        nc.gpsimd.memset(_ap(sb, 64, [[VS, 6], [1, 2]]).bitcast(F32), 0.0)
                        nc.gpsimd.memset(_ap(sb, 64, [[VS, 6], [1, 1]]).bitcast(F32), 1.0)
                        nc.sync.dma_start(out=v_d[p0 + 98 * s:p0 + 98 * s + 98, :],
                                          in_=sb[:])

        # ===== Phase C: rel values =====
        with tc.tile_pool(name="pC", bufs=2) as pC, \
             tc.tile_pool(name="pCr", bufs=2) as pCr, \
             tc.tile_pool(name="pC_ps", bufs=4, space="PSUM") as pC_ps:
            for h in range(NH):
                qTh = pC.tile([HD, NTOK], F32R, tag="qTh", bufs=1)
                nc.sync.dma_start(out=qTh[:], in_=qrel_d[h, 0:64, :])
                relh_sb = pCr.tile([14, NTOK], F32R, tag="relh")
                relw_sb = pCr.tile([14, NTOK], F32R, tag="relw")
                for r in range(14):
                    ps = pC_ps.tile([14, 350], F32, tag="rel_ps")
                    nc.tensor.matmul(ps[:], relm_t[:, r * 14:(r + 1) * 14],
                                     _ap(qTh, r * 14, [[196, 25], [1, 14]]),
                                     start=True, stop=True)
                    dst = _ap(relh_sb, r * 14, [[196, 25], [1, 14]])
                    if r % 2 == 0:
                        nc.scalar.copy(out=dst, in_=ps[:])
                    else:
                        nc.vector.tensor_copy(out=dst, in_=ps[:])
                for c in range(14):
                    ps = pC_ps.tile([14, 350], F32, tag="rel_ps")
                    nc.tensor.matmul(ps[:], relm_t[:, 196 + c * 14:196 + (c + 1) * 14],
                                     _ap(qTh, c, [[196, 25], [14, 14]]),
                                     start=True, stop=True)
                    dst = _ap(relw_sb, c, [[196, 25], [14, 14]])
                    if c % 2 == 0:
                        nc.scalar.copy(out=dst, in_=ps[:])
                    else:
                        nc.vector.tensor_copy(out=dst, in_=ps[:])
                nc.sync.dma_start(out=qrel_d[h, 64:78, :], in_=relh_sb[:])
                nc.sync.dma_start(out=qrel_d[h, 78:92, :], in_=relw_sb[:])
                ia = indA[:]
                ib = indB[:]
                nc.sync.dma_start(
                    out=qrel_d[h, 92:93, 0:4704],
                    in_=bass.AP(tensor=ia.tensor, offset=ia.offset,
                                ap=[ia.ap[0], [0, 12], [1, 392]]))
                nc.sync.dma_start(out=qrel_d[h, 92:93, 4704:4900], in_=indA[:, 0:196])
                nc.sync.dma_start(
                    out=qrel_d[h, 93:94, 0:4704],
                    in_=bass.AP(tensor=ib.tensor, offset=ib.offset,
                                ap=[ib.ap[0], [0, 12], [1, 392]]))
                nc.sync.dma_start(out=qrel_d[h, 93:94, 4704:4900], in_=indB[:, 0:196])

        # ===== Phase D: attention + proj =====
        with tc.tile_pool(name="pD", bufs=4) as pD, \
             tc.tile_pool(name="pDv", bufs=2) as pDv, \
             tc.tile_pool(name="pDa", bufs=2) as pDa, \
             tc.tile_pool(name="pDy", bufs=2) as pDy, \
             tc.tile_pool(name="pDe", bufs=6) as pDe, \
             tc.tile_pool(name="pDs_ps", bufs=3, space="PSUM") as pDs_ps, \
             tc.tile_pool(name="pDo_ps", bufs=2, space="PSUM") as pDo_ps, \
             tc.tile_pool(name="pDz_ps", bufs=1, space="PSUM") as pDz_ps, \
             tc.tile_pool(name="pDp_ps", bufs=1, space="PSUM") as pDp_ps:
            for (p0, plen) in PGROUPS:
                nk = plen // 98
                vt = [pDv.tile([98, NH * VS], F32R, tag=f"vt{s}", name=f"vt{s}") for s in range(nk)]
                for s in range(nk):
                    nc.sync.dma_start(out=vt[s][:],
                                      in_=v_d[p0 + 98 * s:p0 + 98 * (s + 1), :])
                attnT = [pDa.tile([128, 392], F32R, tag=f"attnT{m}", name=f"attnT{m}") for m in range(3)]
                for h in range(NH):
                    kTa = pD.tile([128, 392], F32R, tag="kTa")
                    nc.sync.dma_start(out=kTa[0:64, 0:plen],
                                      in_=kT_d[h * HD:(h + 1) * HD, p0:p0 + plen])
                    nc.gpsimd.tensor_copy(out=kTa[64:128, 0:plen], in_=kpat[:, 0:plen])
                    qTa = pD.tile([128, 392], F32R, tag="qTa")
                    nc.sync.dma_start(out=qTa[0:94, 0:plen],
                                      in_=qrel_d[h, :, p0:p0 + plen])

                    oT = pDo_ps.tile([VS, 392], F32, tag="oT")
                    for s in range(nk):
                        st = pDs_ps.tile([98, 392], F32, tag="st")
                        nc.tensor.matmul(st[:, 0:plen], kTa[:, s * 98:(s + 1) * 98],
                                         qTa[:, 0:plen], start=True, stop=True)
                        et = pDe.tile([98, 392], F32R, tag="et")
                        nc.scalar.activation(out=et[:, 0:plen], in_=st[:, 0:plen],
                                             func=AF.Exp, bias=0.0, scale=1.0)
                        nc.tensor.matmul(oT[:, 0:plen], vt[s][:, h * VS:(h + 1) * VS],
                                         et[:, 0:plen], start=(s == 0),
                                         stop=(s == nk - 1))
                    rz = pD.tile([1, 392], F32, tag="rz")
                    nc.vector.reciprocal(out=rz[:, 0:plen], in_=oT[64:65, 0:plen])
                    rzb = pD.tile([64, 392], F32, tag="rzb")
                    nc.gpsimd.partition_broadcast(rzb[:, 0:plen], rz[:, 0:plen])
                    nc.vector.tensor_tensor(
                        out=attnT[h // 2][(h % 2) * 64:(h % 2) * 64 + 64, 0:plen],
                        in0=oT[0:64, 0:plen], in1=rzb[:, 0:plen], op=ALU.mult)
                yT = []
                for m in range(3):
                    pj = pDp_ps.tile([128, 392], F32, tag="pj", bufs=2)
                    for kc in range(3):
                        nc.tensor.matmul(pj[:, 0:plen],
                                         wp_t[:, kc, m * 128:(m + 1) * 128],
                                         attnT[kc][:, 0:plen],
                                         start=(kc == 0), stop=(kc == 2))
                    yTm = pDy.tile([128, 392], F32R, tag=f"yT{m}")
                    if m == 1:
                        nc.vector.tensor_scalar(out=yTm[:, 0:plen], in0=pj[:, 0:plen],
                                                scalar1=bp_t[:, m:m + 1], scalar2=None,
                                                op0=ALU.add)
                    else:
                        nc.scalar.activation(out=yTm[:, 0:plen], in_=pj[:, 0:plen],
                                             func=AF.Identity, bias=bp_t[:, m:m + 1],
                                             scale=1.0)
                    yT.append(yTm)
                for s in range(nk):
                    pt = pDp_ps.tile([98, DIM], F32R, tag="ytr")
                    for m in range(3):
                        nc.tensor.transpose(pt[:, m * 128:(m + 1) * 128],
                                            yT[m][:, s * 98:(s + 1) * 98], ident[:])
                    ysb = pD.tile([98, DIM], F32, tag="ysb")
                    nc.vector.tensor_copy(out=ysb[:], in_=pt[:].bitcast(F32))
                    w = (p0 + 98 * s) // 196
                    r0 = ((p0 + 98 * s) % 196) // 14
                    i0 = (w // 5) * 14 + r0
                    j0 = (w % 5) * 14
                    ya = y_d.ap()
                    ydst = bass.AP(tensor=ya.tensor, offset=(i0 * HP + j0) * DIM,
                                   ap=[[HP * DIM, 7], [DIM, 14], [1, DIM]])
                    nc.sync.dma_start(out=ydst, in_=ysb[:])

        # ===== Phase E: residual + LN2 + MLP =====
        with tc.tile_pool(name="pE", bufs=3) as pE, \
             tc.tile_pool(name="pEz", bufs=6) as pEz, \
             tc.tile_pool(name="pEh", bufs=2) as pEh, \
             tc.tile_pool(name="pEgG", bufs=2) as pEgG, \
             tc.tile_pool(name="pE_ps", bufs=2, space="PSUM") as pE_ps, \
             tc.tile_pool(name="pE2_ps", bufs=2, space="PSUM") as pE2_ps, \
             tc.tile_pool(name="pE3_ps", bufs=2, space="PSUM") as pE3_ps:
            for g in range(8):
                zts = []
                h2T = [pEh.tile([128, 512], F32R, tag=f"h2T{c}", name=f"h2T{c}") for c in range(3)]
                for tt in range(4):
                    t = g * 4 + tt
                    xt = pE.tile([128, DIM], F32, tag="xe")
                    nc.sync.dma_start(out=xt[:], in_=x_in[t * 128:(t + 1) * 128, :])
                    yt = pE.tile([128, DIM], F32, tag="ye")
                    ya = y_d.ap()
                    nc.sync.dma_start(
                        out=yt[:],
                        in_=bass.AP(tensor=ya.tensor, offset=(2 * t) * HP * DIM,
                                    ap=[[HP * DIM, 2], [DIM, 64], [1, DIM]]))
                    zt = pEz.tile([128, DIM], F32, tag="ze")
                    nc.vector.tensor_tensor(out=zt[:], in0=xt[:], in1=yt[:], op=ALU.add)
                    zts.append(zt)
                    stats = pE.tile([128, 6], F32, tag="st_e")
                    nc.vector.bn_stats(out=stats[:], in_=zt[:])
                    mv = pE.tile([128, 2], F32, tag="mv_e")
                    nc.vector.bn_aggr(out=mv[:], in_=stats[:])
                    rstd = pE.tile([128, 1], F32, tag="rstd_e")
                    nc.scalar.activation(out=rstd[:], in_=mv[:, 1:2], func=AF.Sqrt,
                                         bias=eps_t[:], scale=1.0)
                    nc.vector.reciprocal(out=rstd[:], in_=rstd[:])
                    nmr = pE.tile([128, 1], F32, tag="nmr_e")
                    nc.vector.tensor_scalar_mul(out=nmr[:], in0=mv[:, 0:1], scalar1=-1.0)
                    hn = pE.tile([128, DIM], F32R, tag="hn_e")
                    nc.scalar.activation(out=hn[:], in_=zt[:], func=AF.Identity,
                                         bias=nmr[:], scale=rstd[:])
                    for c in range(3):
                        pt = pE_ps.tile([128, 128], F32R, tag="htr")
                        nc.tensor.transpose(pt[:], hn[:, c * 128:(c + 1) * 128], ident[:])
                        if (tt * 3 + c) % 2 == 0:
                            nc.scalar.copy(out=h2T[c][:, tt * 128:(tt + 1) * 128], in_=pt[:])
                        else:
                            nc.vector.tensor_copy(out=h2T[c][:, tt * 128:(tt + 1) * 128],
                                                  in_=pt[:])
                gt = []
                for m in range(12):
                    ps = pE2_ps.tile([128, 512], F32, tag="fc1")
                    for kc in range(3):
                        nc.tensor.matmul(ps[:], w1_t[:, kc, m * 128:(m + 1) * 128],
                                         h2T[kc][:], start=(kc == 0), stop=(kc == 2))
                    gm = pEgG.tile([128, 512], F32R, tag=f"g{m}")
                    nc.scalar.activation(out=gm[:], in_=ps[:], func=AF.Gelu,
                                         bias=b1_t[:, m:m + 1], scale=1.0)
                    gt.append(gm)
                o2T = []
                for m in range(3):
                    ps = pE3_ps.tile([128, 512], F32, tag="fc2")
                    for kc in range(12):
                        nc.tensor.matmul(ps[:], w2_t[:, kc, m * 128:(m + 1) * 128],
                                         gt[kc][:], start=(kc == 0), stop=(kc == 11))
                    om = pEh.tile([128, 512], F32R, tag=f"o2T{m}")
                    if m == 1:
                        nc.vector.tensor_scalar(out=om[:], in0=ps[:],
                                                scalar1=b2_t[:, m:m + 1], scalar2=None,
                                                op0=ALU.add)
                    else:
                        nc.scalar.activation(out=om[:], in_=ps[:], func=AF.Identity,
                                             bias=b2_t[:, m:m + 1], scale=1.0)
                    o2T.append(om)
                for tt in range(4):
                    t = g * 4 + tt
                    pt = pE_ps.tile([128, DIM], F32R, tag="otr")
                    for c in range(3):
                        nc.tensor.transpose(pt[:, c * 128:(c + 1) * 128],
                                            o2T[c][:, tt * 128:(tt + 1) * 128], ident[:])
                    ot = pE.tile([128, DIM], F32, tag="oe")
                    nc.vector.tensor_tensor(out=ot[:], in0=zts[tt][:], in1=pt[:].bitcast(F32),
                                            op=ALU.add)
                    nc.sync.dma_start(out=out_d[t * 128:(t + 1) * 128, :], in_=ot[:])

    nc.compile()
    return nc


_NC = None


def _get_nc():
    global _NC
    if _NC is None:
        _NC = build_bass()
    return _NC


def _host_prep(inputs):
    f = np.float32
    ln1_w = np.asarray(inputs["ln1_w"], f); ln1_b = np.asarray(inputs["ln1_b"], f)
    qkv_w = np.asarray(inputs["qkv_w"], f); qkv_b = np.asarray(inputs["qkv_b"], f)
    proj_w = np.asarray(inputs["proj_w"], f); proj_b = np.asarray(inputs["proj_b"], f)
    ln2_w = np.asarray(inputs["ln2_w"], f); ln2_b = np.asarray(inputs["ln2_b"], f)
    fc1_w = np.asarray(inputs["fc1_w"], f); fc1_b = np.asarray(inputs["fc1_b"], f)
    fc2_w = np.asarray(inputs["fc2_w"], f); fc2_b = np.asarray(inputs["fc2_b"], f)
    rel_h = np.asarray(inputs["rel_pos_h"], f); rel_w = np.asarray(inputs["rel_pos_w"], f)

    wqk = (ln1_w[:, None] * qkv_w[:, :768]).copy()
    bqk = (ln1_b @ qkv_w[:, :768] + qkv_b[:768]).copy()
    wqk[:, :384] *= SCALE
    bqk[:384] *= SCALE
    wv = (ln1_w[:, None] * qkv_w[:, 768:]).copy()
    bv = ln1_b @ qkv_w[:, 768:] + qkv_b[768:]

    coords = np.arange(WS)[:, None] - np.arange(WS)[None, :] + (WS - 1)
    Rh = rel_h[coords]
    Rw = rel_w[coords]
    rel = np.zeros((HD, 2 * 196), f)
    for r in range(14):
        rel[:, r * 14:(r + 1) * 14] = Rh[r].T / SCALE
    for c in range(14):
        rel[:, 196 + c * 14:196 + (c + 1) * 14] = Rw[c].T / SCALE

    return {
        "wqk": np.ascontiguousarray(wqk, f), "bqk": np.ascontiguousarray(bqk, f),
        "wv": np.ascontiguousarray(wv, f), "rel": rel,
        "wp": np.ascontiguousarray(proj_w, f),
        "bp": np.ascontiguousarray(proj_b + bv @ proj_w, f),
        "w1": np.ascontiguousarray(ln2_w[:, None] * fc1_w, f),
        "b1": np.ascontiguousarray(ln2_b @ fc1_w + fc1_b, f),
        "w2": np.ascontiguousarray(fc2_w, f),
        "b2": np.ascontiguousarray(fc2_b, f),
    }


def kernel(**inputs):
    nc = _get_nc()
    shared = _host_prep(inputs)
    x = np.asarray(inputs["x"], np.float32).reshape(B, NVAL, DIM)
    in_maps = [dict(shared, x=np.ascontiguousarray(x[c])) for c in range(B)]
    res = run_bass_kernel_spmd(nc, in_maps, list(range(B)))
    out = np.stack([res.results[c]["out"] for c in range(B)])
    return out.reshape(B, H, W, DIM)


if __name__ == "__main__":
    build_bass()
    print("build ok")
